# revision 32
# baseline (speedup 1.0000x reference)
"""CrossTransFormer attention kernel for 8x Trainium2 NeuronCores (Bass/Tile).

Problem (per batch b, B=8, C=773, P=4096):
    K = Wk @ Xk + bk            [C, P]
    V = Wv @ Xq + bv            [C, P]
    S[i, j] = sum_c K[c, i] * V[c, j] / sqrt(C)       (i, j over P)
    H = softmax(S, axis=i)
    out[k, j] = sum_i Xk[k, i] * H[i, j]              [C, P]

Sharding: data-parallel over batch, one batch per NeuronCore, no collectives.

Algebraic restructure (saves one full projection + all weight transposes):
    S = Xk^T (Wk^T Wv) Xq + u 1^T + 1 w^T   with u = Xk^T (Wk^T bv);
    the j-indexed w term is constant along the softmax axis i and cancels
    exactly -> dropped.
  GT = Wv^T Wk is computed on the PE with both weights in NATURAL layout,
  w1 = Wk^T bv rides along; both fold into the A-projection
  A = G Xq + w1 1^T.  The A-proj lhsT tiles are zero-padded to c1=896 so
  every projection chain emits full-128-partition PSUM tiles: the staged A
  is then zero-filled in its ragged rows FOR FREE, letting every S matmul
  run K=128.

QT = Xk^T is produced by the DMA XBAR transpose engine (14 batched
dma_start_transpose calls), entirely off the PE.  xk16 c-tile 6 carries an
all-ones row at partition 32, so the transpose plants an all-ones column at
qt col 800 for free; the ragged out-chain then lands softmax sums on PSUM
partition 32 (legal compute-engine base).  No plain SBUF->SBUF HWDGE DMAs
are issued anywhere (XBAR-transpose || SBUF->SBUF DMA is a known HW
deadlock): the w1 row is computed directly into PSUM partition 5 by giving
the bias column M=6 (cols 0..4 zero), and the xq6 ones row comes from a
memset-1.0 + partial overwrite.

Fused phase D (per j-block of 512), everything SBUF-resident:
  A-proj: 7 chains of 7 MMs -> ast[128, 7, 512] fp16 (no DRAM staging).
  S-phase: 32 i-tiles, 7-MM chains into triple-buffered PSUM, ACT exp
  (scale=1/sqrt(C)) into es[128, 32, 512] fp16.
  out-phase: 7 k-tile chains of 32 accumulating MMs; the ragged chain
  (5 data rows + softmax-sum row from the qt ones-column) runs FIRST so
  the reciprocal + partition-broadcast overlap the remaining chains;
  each chain is normalized (DVE) and DMA'd out as it finishes.
"""

import sys

sys.path.insert(0, "/opt/trn_rl_repo")

import numpy as np

import concourse.bacc as bacc
import concourse.mybir as mybir
import concourse.tile as tile
from concourse.bass_utils import run_bass_kernel_spmd

F32 = mybir.dt.float32
F16 = mybir.dt.float16

C = 773
PT = 128
CT = 7  # ceil(773 / 128) chunks of the channel dim
LC = C - (CT - 1) * PT  # 5 rows in the last chunk
JB = 512  # j-block width (one PSUM bank of fp32)
CW = CT * PT  # c1 padded to 896 for the zero-padded A-proj lhsT
QW = 6 * PT + 48  # qt width: 6 full c-tiles + 48-col XBAR tail block


def build(P=4096, n_cores=8):
    NJ = P // JB
    IT = P // PT
    SCALE = float(1.0 / np.sqrt(C))

    nc = bacc.Bacc("TRN2", target_bir_lowering=False, debug=False,
                   num_devices=n_cores)
    Xq = nc.dram_tensor("Xq", [C, P], F32, kind="ExternalInput")
    Xk = nc.dram_tensor("Xk", [C, P], F32, kind="ExternalInput")
    Wk = nc.dram_tensor("Wk", [C, C], F32, kind="ExternalInput")
    bk = nc.dram_tensor("bk", [C], F32, kind="ExternalInput")
    Wv = nc.dram_tensor("Wv", [C, C], F32, kind="ExternalInput")
    bv = nc.dram_tensor("bv", [C], F32, kind="ExternalInput")
    out = nc.dram_tensor("out", [C, P], F32, kind="ExternalOutput")
    del bk  # only enters via a softmax-invariant per-j term

    with tile.TileContext(nc) as tc:
        with tc.tile_pool(name="persist", bufs=1) as persist:
            # Xk fp16 resident, natural [c, p] layout: lhsT tiles for S.
            # Tile 6: rows 0..4 = ragged data, row 32 = all-ones (becomes
            # the qt ones-column via the XBAR transpose; contributes 0 to S
            # because ast tile-6 rows 5..127 are zero), rest zeros.
            xk16 = persist.tile([PT, CT, P], F16)
            # exp(S) for one j-block, [i-in-tile, it, j]
            es = persist.tile([PT, IT, JB], F16)
            # GT = Wv^T Wk [c2-part, ct2, c1] fp16, c1 zero-padded to 896
            g16 = persist.tile([PT, CT, CW], F16)
            # packed ragged lhsT: rows 0..4 = GT c2-ragged rows, row 5 = w1
            g6 = persist.tile([8, CW], F16)

            # PE warmup: dummy matmuls so the HAM clock-gate opens
            # (4/8 -> 8/8) while the first DMAs are in flight, and the
            # exp activation table loads before the main loop.  warm is
            # memset on DVE (gpsimd takes ~8us to boot).
            wsb = tc.alloc_tile_pool(name="wsb", bufs=1)
            warm = wsb.tile([PT, JB], F16)
            nc.vector.memset(warm[:, :], 0.0)
            with tc.tile_pool(name="pswarm", bufs=4, space="PSUM") as pswarm:
                for i in range(52):
                    wps = pswarm.tile([PT, JB], F32, tag="wps",
                                      name=f"wps{i}")
                    nc.tensor.matmul(wps[:, :], warm[:, :PT], warm[:, :],
                                     start=True, stop=True,
                                     skip_group_check=True)
                wexp = wsb.tile([1, 16], F32)
                nc.scalar.activation(wexp[:], wps[:1, :16],
                                     mybir.ActivationFunctionType.Exp,
                                     scale=1.0)
            wsb.release()

            # zero-pad fills on gpsimd (consumers run ~15us+, gpsimd boot
            # overlaps).  The xk16 tile-6 fills are issued on DVE but only
            # AFTER the W casts below, so the G-phase critical path is not
            # delayed; their first consumer is the jc-0 tail cast (~20us).
            nc.gpsimd.memset(g16[:, :, :], 0.0)
            nc.gpsimd.memset(g6[:, :], 0.0)

            # ---- Phase G: GT = Wv^T Wk and w1 = Wk^T bv on the PE ----
            with (
                tc.tile_pool(name="wstg", bufs=12) as wstg,
                tc.tile_pool(name="wtlp", bufs=2) as wtlp,
                tc.tile_pool(name="wload", bufs=1) as wload,
                tc.tile_pool(name="psg", bufs=4, space="PSUM") as psg,
            ):
                wk16 = wload.tile([PT, CT, C], F16, tag="wk16")
                wv16 = wload.tile([PT, CT, C], F16, tag="wv16")
                # bias columns, M=6 per o-tile: cols 0..4 zero, col 5 = bv
                # chunk -> the w1 chain emits w1 directly on PSUM row 5.
                bvcol = wload.tile([PT, CT, 6], F16, tag="bvcol")
                # per-chunk W loads through a 6-deep ring: 12 concurrent
                # DMAs pull ~250 GB/s aggregate (one queue sustains only
                # ~20 GB/s), landing W in ~12us so phase G starts early.
                for Wsrc, dst in ((Wk, wk16), (Wv, wv16)):
                    for ct in range(CT - 1):
                        ws = wstg.tile([PT, C], F32, tag="wstage")
                        nc.sync.dma_start(
                            ws[:, :], Wsrc[ct * PT:(ct + 1) * PT, :])
                        nc.vector.tensor_copy(dst[:, ct, :], ws[:, :])
                    wt = wtlp.tile([8, C], F32, tag="wtail")
                    nc.sync.dma_start(wt[:LC, :], Wsrc[(CT - 1) * PT:C, :])
                    nc.vector.tensor_copy(dst[:LC, CT - 1, :], wt[:LC, :])
                # deferred DVE fills (after the W casts in DVE program
                # order).  xk16 tile 6: rows 0..4 = ragged data (cast in
                # phase B), row 32 = all-ones -> qt ones-column via XBAR.
                nc.vector.memset(xk16[:, CT - 1, :], 0.0)
                nc.vector.memset(xk16[32:33, CT - 1, :], 1.0)
                nc.vector.memset(bvcol[:, :, :], 0.0)
                # bv chunks into bvcol[:, ot, 5] on the gpsimd software
                # queue (DRAM->SBUF, cast f32->f16)
                for ot in range(CT - 1):
                    nc.gpsimd.dma_start(
                        bvcol[:, ot, 5:6], bv[ot * PT:(ot + 1) * PT, None])
                nc.gpsimd.dma_start(bvcol[:LC, CT - 1, 5:6],
                                    bv[(CT - 1) * PT:C, None])
                # GT tiles: [c2-tile, c1-chunk], contract over o (7 tiles)
                for ct2 in range(CT):
                    pc2 = PT if ct2 < CT - 1 else LC
                    for h, (j0, j1) in enumerate(((0, JB), (JB, C))):
                        ps = psg.tile([PT, JB], F32, tag="psg")
                        for ot in range(CT):
                            po = PT if ot < CT - 1 else LC
                            nc.tensor.matmul(
                                ps[:pc2, :j1 - j0],
                                wv16[:po, ot, ct2 * PT:ct2 * PT + pc2],
                                wk16[:po, ot, j0:j1],
                                start=(ot == 0),
                                stop=(ot == CT - 1),
                            )
                        # evacuate on the idle ACT engine: DVE is busy
                        # with W/Xk casts and would stall the G chains
                        nc.scalar.activation(
                            g16[:pc2, ct2, j0:j1], ps[:pc2, :j1 - j0],
                            mybir.ActivationFunctionType.Copy, scale=1.0)
                # w1 row: lhsT = bvcol (M=6, cols 0..4 zero) -> psum rows
                # 0..4 zero, row 5 = w1.  Copy rows 0..5 into g6 FIRST,
                # then overwrite rows 0..4 with the GT ragged rows (WAW
                # dep keeps the order).
                for h, (j0, j1) in enumerate(((0, JB), (JB, C))):
                    ps = psg.tile([8, JB], F32, tag="psw")
                    for ot in range(CT):
                        po = PT if ot < CT - 1 else LC
                        nc.tensor.matmul(
                            ps[:6, :j1 - j0],
                            bvcol[:po, ot, :],
                            wk16[:po, ot, j0:j1],
                            start=(ot == 0),
                            stop=(ot == CT - 1),
                        )
                    nc.scalar.activation(
                        g6[:6, j0:j1], ps[:6, :j1 - j0],
                        mybir.ActivationFunctionType.Copy, scale=1.0)
                nc.scalar.activation(
                    g6[:LC, :C], g16[:LC, CT - 1, :C],
                    mybir.ActivationFunctionType.Copy, scale=1.0)

            # QT pool reuses the space wload released.  qt[i, it, c]:
            # cols 0..767 from c-tiles 0..5, cols 768..815 from the 48-row
            # tail block (data rows 0..4 -> cols 768..772, ones row 32 ->
            # col 800, zeros elsewhere).
            qtp = tc.alloc_tile_pool(name="qtp", bufs=1)
            qt = qtp.tile([PT, IT, QW], F16)

            # pools that span phases B and D.  xfp is a deep per-chunk
            # staging ring: input DMA throughput scales with the number of
            # in-flight dma_starts (~20 GB/s per queue), so 7 concurrent
            # 256 KB chunk loads pull ~2x the aggregate bandwidth of the
            # 2-deep batched scheme.
            xqp = tc.alloc_tile_pool(name="xqp", bufs=2)
            xfp = tc.alloc_tile_pool(name="xfp", bufs=7)
            xtp = tc.alloc_tile_pool(name="xtp", bufs=2)

            def load_xq(jb):
                js = slice(jb * JB, (jb + 1) * JB)
                xq16 = xqp.tile([PT, CT, JB], F16, tag="xq16",
                                name=f"xq16_{jb}")
                for ct in range(CT - 1):
                    xf = xfp.tile([PT, JB], F32, tag="xstage",
                                  name=f"xqf{jb}_{ct}")
                    nc.sync.dma_start(
                        xf[:, :], Xq[ct * PT:(ct + 1) * PT, js])
                    nc.vector.tensor_copy(xq16[:, ct, :], xf[:, :])
                xt = xtp.tile([8, JB], F32, tag="xtail", name=f"xqt{jb}")
                nc.sync.dma_start(xt[:LC, :], Xq[(CT - 1) * PT:C, js])
                nc.vector.tensor_copy(xq16[:LC, CT - 1, :], xt[:LC, :])
                # packed ragged rhs: memset 1.0 (row 5 stays ones, rows
                # 6..7 hit zero g6 rows), rows 0..4 overwritten with the
                # Xq c2-ragged rows.
                xq6 = xqp.tile([8, JB], F16, tag="xq6", name=f"xq6_{jb}")
                nc.vector.memset(xq6[:, :], 1.0)
                nc.vector.tensor_copy(xq6[:LC, :], xq16[:LC, CT - 1, :])
                return xq16, xq6

            # ---- Phase B: stream Xk -> resident fp16 (DMA + DVE only).
            # jc order 1..7 then 0: during the W-load window only jc1
            # competes for DMA engines (W lands ~16us -> G starts early),
            # and the late jc0 is harmless because the S-phase of block 0
            # consumes i-tiles in order 4..31 then 0..3. ----
            xq_next = None
            for jc in tuple(range(1, NJ)) + (0,):
                js = slice(jc * JB, (jc + 1) * JB)
                for ct in range(CT - 1):
                    xf = xfp.tile([PT, JB], F32, tag="xstage",
                                  name=f"xkf{jc}_{ct}")
                    nc.sync.dma_start(
                        xf[:, :], Xk[ct * PT:(ct + 1) * PT, js])
                    nc.vector.tensor_copy(xk16[:, ct, js], xf[:, :])
                xt = xtp.tile([8, JB], F32, tag="xtail", name=f"xkt{jc}")
                nc.sync.dma_start(xt[:LC, :], Xk[(CT - 1) * PT:C, js])
                nc.vector.tensor_copy(xk16[:LC, CT - 1, js], xt[:LC, :])
                if jc == 1:
                    xq_next = load_xq(0)

            # prefetch xq block 1 BEFORE the XBAR gens occupy the sync
            # queue (they block on cast semaphores until ~2/3 through
            # phase B, which would delay block 1's A-projection)
            xq_pre = load_xq(1)

            # ---- QT: batched XBAR transposes (off the PE entirely) ----
            # Two halves per c-tile so the first half can fire as soon as
            # jc 0..3 have landed.  out[p, t, f] = in[f, 128*t + p].
            # half 1 first (its jc 4..7 casts land well before the
            # deliberately-late jc 0); within half 0, ct 6 first so the
            # ragged out-chain's qt tail block is ready earliest.
            for half, ct_order in ((1, range(CT)),
                                   (0, (CT - 1,) + tuple(range(CT - 1)))):
                hp = slice(half * (P // 2), (half + 1) * (P // 2))
                ht = slice(half * (IT // 2), (half + 1) * (IT // 2))
                for ct in ct_order:
                    pc = PT if ct < CT - 1 else 48
                    nc.sync.dma_start_transpose(
                        qt[:, ht, ct * PT:ct * PT + pc],
                        xk16[:pc, ct, hp],
                    )

            # ---- Phase D: fused A-projection + attention main loop ----
            with (
                tc.tile_pool(name="astp", bufs=2) as astp,
                tc.tile_pool(name="op", bufs=2) as op,
                tc.tile_pool(name="rp", bufs=1) as rp,
                tc.tile_pool(name="psA", bufs=2, space="PSUM") as psA,
                tc.tile_pool(name="psS", bufs=3, space="PSUM") as psS,
                tc.tile_pool(name="psO", bufs=3, space="PSUM") as psO,
            ):
                for jb in range(NJ):
                    js = slice(jb * JB, (jb + 1) * JB)
                    xq16, xq6 = xq_next
                    if jb == 0:
                        xq_next = xq_pre
                    elif jb < NJ - 1:
                        xq_next = load_xq(jb + 1)

                    # A-proj: A[:, jblock] = G @ Xq + w1 (ragged K=6 MM
                    # carries both the c2 tail and the bias row); the
                    # zero-padded lhsT makes all 128 psum rows valid
                    ast = astp.tile([PT, CT, JB], F16, tag="ast",
                                    name=f"ast{jb}")
                    for ot in range(CT):
                        ps = psA.tile([PT, JB], F32, tag="a",
                                      name=f"a{jb}_{ot}")
                        for ct2 in range(CT - 1):
                            nc.tensor.matmul(
                                ps[:, :],
                                g16[:, ct2, ot * PT:(ot + 1) * PT],
                                xq16[:, ct2, :],
                                start=(ct2 == 0),
                                stop=False,
                                skip_group_check=True,
                            )
                        nc.tensor.matmul(
                            ps[:, :],
                            g6[:LC + 1, ot * PT:(ot + 1) * PT],
                            xq6[:LC + 1, :],
                            start=False,
                            stop=True,
                            skip_group_check=True,
                        )
                        nc.any.tensor_copy(ast[:, ot, :], ps[:, :])

                    # S-phase: 32 chains of 7 K=128 MMs, exp into es.
                    # i-tiles 4..31 first: block 0's tiles 0..3 depend on
                    # the deliberately-late jc-0 load.
                    for t in tuple(range(4, IT)) + (0, 1, 2, 3):
                        ts = slice(t * PT, (t + 1) * PT)
                        ps_s = psS.tile([PT, JB], F32, tag="s",
                                        name=f"s{jb}_{t}")
                        for ct in range(CT):
                            nc.tensor.matmul(
                                ps_s[:, :],
                                xk16[:, ct, ts],
                                ast[:, ct, :],
                                start=(ct == 0),
                                stop=(ct == CT - 1),
                                skip_group_check=True,
                            )
                        nc.scalar.activation(
                            es[:, t, :], ps_s[:],
                            mybir.ActivationFunctionType.Exp, scale=SCALE,
                        )

                    # out-phase: chain kt=0 FIRST (its early MMs only need
                    # the early es tiles, hiding the exp tail), then the
                    # ragged chain (data rows 768..772 + sums row from the
                    # qt ones column) so the reciprocal + broadcast still
                    # overlap the remaining chains.  kt=0's normalization
                    # is deferred until bc exists (PSUM bank held).
                    bc = None
                    held = None

                    def normalize(kt, ps_o):
                        mo = LC if kt == CT - 1 else PT
                        osb = op.tile([PT, JB], F32, tag="osb",
                                      name=f"osb{jb}_{kt}")
                        nc.vector.tensor_mul(
                            out=osb[:mo, :], in0=ps_o[:mo, :],
                            in1=bc[:mo, :],
                        )
                        nc.sync.dma_start(
                            out[kt * PT:kt * PT + mo, js], osb[:mo, :])

                    for kt in (0, CT - 1) + tuple(range(1, CT - 1)):
                        mk = 33 if kt == CT - 1 else PT
                        ps_o = psO.tile([PT, JB], F32, tag="o",
                                        name=f"o{jb}_{kt}")
                        # accumulate i-tiles 4..31 first to match the
                        # S-phase exp order (block 0's es 0..3 land last)
                        for n, t in enumerate(tuple(range(4, IT)) +
                                              (0, 1, 2, 3)):
                            nc.tensor.matmul(
                                ps_o[:mk, :],
                                qt[:, t, kt * PT:kt * PT + mk],
                                es[:, t, :],
                                start=(n == 0),
                                stop=(n == IT - 1),
                                skip_group_check=True,
                            )
                        if kt == CT - 1:
                            # fp16 1/den costs ~5e-4 relative error on the
                            # output, far under the 2e-2 gate; halves SBUF
                            recip = rp.tile([1, JB], F16, tag="recip")
                            with nc.allow_low_precision(
                                    reason="fp16 softmax denom recip"):
                                nc.vector.reciprocal(
                                    recip[:], ps_o[32:33, :])
                            bc = rp.tile([PT, JB], F16, tag="bc")
                            nc.gpsimd.partition_broadcast(bc[:], recip[:])
                        if bc is None:
                            held = (kt, ps_o)
                            continue
                        normalize(kt, ps_o)
                        if held is not None:
                            normalize(*held)
                            held = None

            xtp.release()
            xfp.release()
            xqp.release()
            qtp.release()

    nc.compile()
    return nc


_CACHE = {}


def _get_program(P=4096, n_cores=8):
    key = (P, n_cores)
    if key not in _CACHE:
        _CACHE[key] = build(P, n_cores)
    return _CACHE[key]


def _run(inputs, trace=False, **kw):
    nc = _get_program()
    Xq = np.asarray(inputs["Xq"], dtype=np.float32)
    Xk = np.asarray(inputs["Xk"], dtype=np.float32)
    Wk = np.ascontiguousarray(np.asarray(inputs["Wk"], dtype=np.float32))
    bkv = np.ascontiguousarray(np.asarray(inputs["bk"], dtype=np.float32))
    Wv = np.ascontiguousarray(np.asarray(inputs["Wv"], dtype=np.float32))
    bvv = np.ascontiguousarray(np.asarray(inputs["bv"], dtype=np.float32))
    B = Xq.shape[0]
    in_maps = [
        {
            "Xq": np.ascontiguousarray(Xq[b]),
            "Xk": np.ascontiguousarray(Xk[b]),
            "Wk": Wk,
            "bk": bkv,
            "Wv": Wv,
            "bv": bvv,
        }
        for b in range(B)
    ]
    res = run_bass_kernel_spmd(nc, in_maps, list(range(B)), trace=trace, **kw)
    outs = np.stack([res.results[b]["out"] for b in range(B)], axis=0)
    return outs.astype(np.float32), res


def kernel(**inputs):
    outs, _ = _run(inputs)
    return outs


# revision 34
# speedup vs baseline: 1.0038x; 1.0038x over previous
"""CrossTransFormer attention kernel for 8x Trainium2 NeuronCores (Bass/Tile).

Problem (per batch b, B=8, C=773, P=4096):
    K = Wk @ Xk + bk            [C, P]
    V = Wv @ Xq + bv            [C, P]
    S[i, j] = sum_c K[c, i] * V[c, j] / sqrt(C)       (i, j over P)
    H = softmax(S, axis=i)
    out[k, j] = sum_i Xk[k, i] * H[i, j]              [C, P]

Sharding: data-parallel over batch, one batch per NeuronCore, no collectives.

Algebraic restructure (saves one full projection + all weight transposes):
    S = Xk^T (Wk^T Wv) Xq + u 1^T + 1 w^T   with u = Xk^T (Wk^T bv);
    the j-indexed w term is constant along the softmax axis i and cancels
    exactly -> dropped.
  GT = Wv^T Wk is computed on the PE with both weights in NATURAL layout,
  w1 = Wk^T bv rides along; both fold into the A-projection
  A = G Xq + w1 1^T.  The A-proj lhsT tiles are zero-padded to c1=896 so
  every projection chain emits full-128-partition PSUM tiles: the staged A
  is then zero-filled in its ragged rows FOR FREE, letting every S matmul
  run K=128.

QT = Xk^T is produced by the DMA XBAR transpose engine (14 batched
dma_start_transpose calls), entirely off the PE.  xk16 c-tile 6 carries an
all-ones row at partition 32, so the transpose plants an all-ones column at
qt col 800 for free; the ragged out-chain then lands softmax sums on PSUM
partition 32 (legal compute-engine base).  No plain SBUF->SBUF HWDGE DMAs
are issued anywhere (XBAR-transpose || SBUF->SBUF DMA is a known HW
deadlock): the w1 row is computed directly into PSUM partition 5 by giving
the bias column M=6 (cols 0..4 zero), and the xq6 ones row comes from a
memset-1.0 + partial overwrite.

Fused phase D (per j-block of 512), everything SBUF-resident:
  A-proj: 7 chains of 7 MMs -> ast[128, 7, 512] fp16 (no DRAM staging).
  S-phase: 32 i-tiles, 7-MM chains into triple-buffered PSUM, ACT exp
  (scale=1/sqrt(C)) into es[128, 32, 512] fp16.
  out-phase: 7 k-tile chains of 32 accumulating MMs; the ragged chain
  (5 data rows + softmax-sum row from the qt ones-column) runs FIRST so
  the reciprocal + partition-broadcast overlap the remaining chains;
  each chain is normalized (DVE) and DMA'd out as it finishes.
"""

import sys

sys.path.insert(0, "/opt/trn_rl_repo")

import numpy as np

import concourse.bacc as bacc
import concourse.mybir as mybir
import concourse.tile as tile
from concourse.bass_utils import run_bass_kernel_spmd

F32 = mybir.dt.float32
F16 = mybir.dt.float16

C = 773
PT = 128
CT = 7  # ceil(773 / 128) chunks of the channel dim
LC = C - (CT - 1) * PT  # 5 rows in the last chunk
JB = 512  # j-block width (one PSUM bank of fp32)
CW = CT * PT  # c1 padded to 896 for the zero-padded A-proj lhsT
QW = 6 * PT + 48  # qt width: 6 full c-tiles + 48-col XBAR tail block


def build(P=4096, n_cores=8):
    NJ = P // JB
    IT = P // PT
    SCALE = float(1.0 / np.sqrt(C))

    nc = bacc.Bacc("TRN2", target_bir_lowering=False, debug=False,
                   num_devices=n_cores)
    Xq = nc.dram_tensor("Xq", [C, P], F32, kind="ExternalInput")
    Xk = nc.dram_tensor("Xk", [C, P], F32, kind="ExternalInput")
    Wk = nc.dram_tensor("Wk", [C, C], F32, kind="ExternalInput")
    bk = nc.dram_tensor("bk", [C], F32, kind="ExternalInput")
    Wv = nc.dram_tensor("Wv", [C, C], F32, kind="ExternalInput")
    bv = nc.dram_tensor("bv", [C], F32, kind="ExternalInput")
    out = nc.dram_tensor("out", [C, P], F32, kind="ExternalOutput")
    del bk  # only enters via a softmax-invariant per-j term

    with tile.TileContext(nc) as tc:
        with tc.tile_pool(name="persist", bufs=1) as persist:
            # Xk fp16 resident, natural [c, p] layout: lhsT tiles for S.
            # Tile 6: rows 0..4 = ragged data, row 32 = all-ones (becomes
            # the qt ones-column via the XBAR transpose; contributes 0 to S
            # because ast tile-6 rows 5..127 are zero), rest zeros.
            xk16 = persist.tile([PT, CT, P], F16)
            # exp(S) for one j-block, [i-in-tile, it, j]
            es = persist.tile([PT, IT, JB], F16)
            # GT = Wv^T Wk [c2-part, ct2, c1] fp16, c1 zero-padded to 896
            g16 = persist.tile([PT, CT, CW], F16)
            # packed ragged lhsT: rows 0..4 = GT c2-ragged rows, row 5 = w1
            g6 = persist.tile([8, CW], F16)

            # PE warmup: dummy matmuls so the HAM clock-gate opens
            # (4/8 -> 8/8) while the first DMAs are in flight, and the
            # exp activation table loads before the main loop.  warm is
            # memset on DVE (gpsimd takes ~8us to boot).
            wsb = tc.alloc_tile_pool(name="wsb", bufs=1)
            warm = wsb.tile([PT, JB], F16)
            nc.vector.memset(warm[:, :], 0.0)
            with tc.tile_pool(name="pswarm", bufs=4, space="PSUM") as pswarm:
                for i in range(94):
                    wps = pswarm.tile([PT, JB], F32, tag="wps",
                                      name=f"wps{i}")
                    nc.tensor.matmul(wps[:, :], warm[:, :PT], warm[:, :],
                                     start=True, stop=True,
                                     skip_group_check=True)
                wexp = wsb.tile([1, 16], F32)
                nc.scalar.activation(wexp[:], wps[:1, :16],
                                     mybir.ActivationFunctionType.Exp,
                                     scale=1.0)
            wsb.release()

            # zero-pad fills on gpsimd (consumers run ~15us+, gpsimd boot
            # overlaps).  The xk16 tile-6 fills are issued on DVE but only
            # AFTER the W casts below, so the G-phase critical path is not
            # delayed; their first consumer is the jc-0 tail cast (~20us).
            nc.gpsimd.memset(g16[:, :, :], 0.0)
            nc.gpsimd.memset(g6[:, :], 0.0)

            # ---- Phase G: GT = Wv^T Wk and w1 = Wk^T bv on the PE ----
            with (
                tc.tile_pool(name="wstg", bufs=6) as wstg,
                tc.tile_pool(name="wtlp", bufs=2) as wtlp,
                tc.tile_pool(name="wload", bufs=1) as wload,
                tc.tile_pool(name="psg", bufs=4, space="PSUM") as psg,
            ):
                wk16 = wload.tile([PT, CT, C], F16, tag="wk16")
                wv16 = wload.tile([PT, CT, C], F16, tag="wv16")
                # bias columns, M=6 per o-tile: cols 0..4 zero, col 5 = bv
                # chunk -> the w1 chain emits w1 directly on PSUM row 5.
                bvcol = wload.tile([PT, CT, 6], F16, tag="bvcol")
                # per-chunk W loads through a 6-deep ring: 12 concurrent
                # DMAs pull ~250 GB/s aggregate (one queue sustains only
                # ~20 GB/s), landing W in ~12us so phase G starts early.
                for Wsrc, dst in ((Wk, wk16), (Wv, wv16)):
                    for ct in range(CT - 1):
                        ws = wstg.tile([PT, C], F32, tag="wstage")
                        nc.sync.dma_start(
                            ws[:, :], Wsrc[ct * PT:(ct + 1) * PT, :])
                        nc.vector.tensor_copy(dst[:, ct, :], ws[:, :])
                    wt = wtlp.tile([8, C], F32, tag="wtail")
                    nc.sync.dma_start(wt[:LC, :], Wsrc[(CT - 1) * PT:C, :])
                    nc.vector.tensor_copy(dst[:LC, CT - 1, :], wt[:LC, :])
                # deferred DVE fills (after the W casts in DVE program
                # order).  xk16 tile 6: rows 0..4 = ragged data (cast in
                # phase B), row 32 = all-ones -> qt ones-column via XBAR.
                nc.vector.memset(xk16[:, CT - 1, :], 0.0)
                nc.vector.memset(xk16[32:33, CT - 1, :], 1.0)
                nc.vector.memset(bvcol[:, :, :], 0.0)
                # bv chunks into bvcol[:, ot, 5] on the gpsimd software
                # queue (DRAM->SBUF, cast f32->f16)
                for ot in range(CT - 1):
                    nc.gpsimd.dma_start(
                        bvcol[:, ot, 5:6], bv[ot * PT:(ot + 1) * PT, None])
                nc.gpsimd.dma_start(bvcol[:LC, CT - 1, 5:6],
                                    bv[(CT - 1) * PT:C, None])
                # GT tiles: [c2-tile, c1-chunk], contract over o (7 tiles)
                for ct2 in range(CT):
                    pc2 = PT if ct2 < CT - 1 else LC
                    for h, (j0, j1) in enumerate(((0, JB), (JB, C))):
                        ps = psg.tile([PT, JB], F32, tag="psg")
                        for ot in range(CT):
                            po = PT if ot < CT - 1 else LC
                            nc.tensor.matmul(
                                ps[:pc2, :j1 - j0],
                                wv16[:po, ot, ct2 * PT:ct2 * PT + pc2],
                                wk16[:po, ot, j0:j1],
                                start=(ot == 0),
                                stop=(ot == CT - 1),
                            )
                        # evacuate on the idle ACT engine: DVE is busy
                        # with W/Xk casts and would stall the G chains
                        nc.scalar.activation(
                            g16[:pc2, ct2, j0:j1], ps[:pc2, :j1 - j0],
                            mybir.ActivationFunctionType.Copy, scale=1.0)
                # w1 row: lhsT = bvcol (M=6, cols 0..4 zero) -> psum rows
                # 0..4 zero, row 5 = w1.  Copy rows 0..5 into g6 FIRST,
                # then overwrite rows 0..4 with the GT ragged rows (WAW
                # dep keeps the order).
                for h, (j0, j1) in enumerate(((0, JB), (JB, C))):
                    ps = psg.tile([8, JB], F32, tag="psw")
                    for ot in range(CT):
                        po = PT if ot < CT - 1 else LC
                        nc.tensor.matmul(
                            ps[:6, :j1 - j0],
                            bvcol[:po, ot, :],
                            wk16[:po, ot, j0:j1],
                            start=(ot == 0),
                            stop=(ot == CT - 1),
                        )
                    nc.scalar.activation(
                        g6[:6, j0:j1], ps[:6, :j1 - j0],
                        mybir.ActivationFunctionType.Copy, scale=1.0)
                nc.scalar.activation(
                    g6[:LC, :C], g16[:LC, CT - 1, :C],
                    mybir.ActivationFunctionType.Copy, scale=1.0)

            # QT pool reuses the space wload released.  qt[i, it, c]:
            # cols 0..767 from c-tiles 0..5, cols 768..815 from the 48-row
            # tail block (data rows 0..4 -> cols 768..772, ones row 32 ->
            # col 800, zeros elsewhere).
            qtp = tc.alloc_tile_pool(name="qtp", bufs=1)
            qt = qtp.tile([PT, IT, QW], F16)

            # pools that span phases B and D.  xfp is a deep per-chunk
            # staging ring: input DMA throughput scales with the number of
            # in-flight dma_starts (~20 GB/s per queue), so 7 concurrent
            # 256 KB chunk loads pull ~2x the aggregate bandwidth of the
            # 2-deep batched scheme.
            xqp = tc.alloc_tile_pool(name="xqp", bufs=2)
            xfp = tc.alloc_tile_pool(name="xfp", bufs=7)
            xtp = tc.alloc_tile_pool(name="xtp", bufs=2)

            def load_xq(jb):
                js = slice(jb * JB, (jb + 1) * JB)
                xq16 = xqp.tile([PT, CT, JB], F16, tag="xq16",
                                name=f"xq16_{jb}")
                for ct in range(CT - 1):
                    xf = xfp.tile([PT, JB], F32, tag="xstage",
                                  name=f"xqf{jb}_{ct}")
                    nc.sync.dma_start(
                        xf[:, :], Xq[ct * PT:(ct + 1) * PT, js])
                    nc.vector.tensor_copy(xq16[:, ct, :], xf[:, :])
                xt = xtp.tile([8, JB], F32, tag="xtail", name=f"xqt{jb}")
                nc.sync.dma_start(xt[:LC, :], Xq[(CT - 1) * PT:C, js])
                nc.vector.tensor_copy(xq16[:LC, CT - 1, :], xt[:LC, :])
                # packed ragged rhs: memset 1.0 (row 5 stays ones, rows
                # 6..7 hit zero g6 rows), rows 0..4 overwritten with the
                # Xq c2-ragged rows.
                xq6 = xqp.tile([8, JB], F16, tag="xq6", name=f"xq6_{jb}")
                nc.vector.memset(xq6[:, :], 1.0)
                nc.vector.tensor_copy(xq6[:LC, :], xq16[:LC, CT - 1, :])
                return xq16, xq6

            # ---- Phase B: stream Xk -> resident fp16 (DMA + DVE only);
            # xq block 0 is interleaved after jc 0 so the Xk stream (the
            # long pole for S0) starts first. ----
            xq_next = None
            for jc in range(NJ):
                js = slice(jc * JB, (jc + 1) * JB)
                for ct in range(CT - 1):
                    xf = xfp.tile([PT, JB], F32, tag="xstage",
                                  name=f"xkf{jc}_{ct}")
                    nc.sync.dma_start(
                        xf[:, :], Xk[ct * PT:(ct + 1) * PT, js])
                    nc.vector.tensor_copy(xk16[:, ct, js], xf[:, :])
                xt = xtp.tile([8, JB], F32, tag="xtail", name=f"xkt{jc}")
                nc.sync.dma_start(xt[:LC, :], Xk[(CT - 1) * PT:C, js])
                nc.vector.tensor_copy(xk16[:LC, CT - 1, js], xt[:LC, :])
                if jc == 1:
                    xq_next = load_xq(0)

            # prefetch xq block 1 BEFORE the XBAR gens occupy the sync
            # queue (they block on cast semaphores until ~2/3 through
            # phase B, which would delay block 1's A-projection)
            xq_pre = load_xq(1)

            # ---- QT: batched XBAR transposes (off the PE entirely) ----
            # Two halves per c-tile so the first half can fire as soon as
            # jc 0..3 have landed.  out[p, t, f] = in[f, 128*t + p].
            for half in (0, 1):
                hp = slice(half * (P // 2), (half + 1) * (P // 2))
                ht = slice(half * (IT // 2), (half + 1) * (IT // 2))
                for ct in range(CT):
                    pc = PT if ct < CT - 1 else 48
                    nc.sync.dma_start_transpose(
                        qt[:, ht, ct * PT:ct * PT + pc],
                        xk16[:pc, ct, hp],
                    )

            # ---- Phase D: fused A-projection + attention main loop ----
            with (
                tc.tile_pool(name="astp", bufs=2) as astp,
                tc.tile_pool(name="op", bufs=2) as op,
                tc.tile_pool(name="rp", bufs=1) as rp,
                tc.tile_pool(name="psA", bufs=2, space="PSUM") as psA,
                tc.tile_pool(name="psS", bufs=3, space="PSUM") as psS,
                tc.tile_pool(name="psO", bufs=3, space="PSUM") as psO,
            ):
                for jb in range(NJ):
                    js = slice(jb * JB, (jb + 1) * JB)
                    xq16, xq6 = xq_next
                    if jb == 0:
                        xq_next = xq_pre
                    elif jb < NJ - 1:
                        xq_next = load_xq(jb + 1)

                    # A-proj: A[:, jblock] = G @ Xq + w1 (ragged K=6 MM
                    # carries both the c2 tail and the bias row); the
                    # zero-padded lhsT makes all 128 psum rows valid
                    ast = astp.tile([PT, CT, JB], F16, tag="ast",
                                    name=f"ast{jb}")
                    for ot in range(CT):
                        ps = psA.tile([PT, JB], F32, tag="a",
                                      name=f"a{jb}_{ot}")
                        for ct2 in range(CT - 1):
                            nc.tensor.matmul(
                                ps[:, :],
                                g16[:, ct2, ot * PT:(ot + 1) * PT],
                                xq16[:, ct2, :],
                                start=(ct2 == 0),
                                stop=False,
                                skip_group_check=True,
                            )
                        nc.tensor.matmul(
                            ps[:, :],
                            g6[:LC + 1, ot * PT:(ot + 1) * PT],
                            xq6[:LC + 1, :],
                            start=False,
                            stop=True,
                            skip_group_check=True,
                        )
                        nc.any.tensor_copy(ast[:, ot, :], ps[:, :])

                    # S-phase: 32 chains of 7 K=128 MMs, exp into es
                    for t in range(IT):
                        ts = slice(t * PT, (t + 1) * PT)
                        ps_s = psS.tile([PT, JB], F32, tag="s",
                                        name=f"s{jb}_{t}")
                        for ct in range(CT):
                            nc.tensor.matmul(
                                ps_s[:, :],
                                xk16[:, ct, ts],
                                ast[:, ct, :],
                                start=(ct == 0),
                                stop=(ct == CT - 1),
                                skip_group_check=True,
                            )
                        nc.scalar.activation(
                            es[:, t, :], ps_s[:],
                            mybir.ActivationFunctionType.Exp, scale=SCALE,
                        )

                    # out-phase: chain kt=0 FIRST (its early MMs only need
                    # the early es tiles, hiding the exp tail), then the
                    # ragged chain (data rows 768..772 + sums row from the
                    # qt ones column) so the reciprocal + broadcast still
                    # overlap the remaining chains.  kt=0's normalization
                    # is deferred until bc exists (PSUM bank held).
                    bc = None
                    held = None

                    def normalize(kt, ps_o):
                        mo = LC if kt == CT - 1 else PT
                        osb = op.tile([PT, JB], F32, tag="osb",
                                      name=f"osb{jb}_{kt}")
                        nc.vector.tensor_mul(
                            out=osb[:mo, :], in0=ps_o[:mo, :],
                            in1=bc[:mo, :],
                        )
                        nc.sync.dma_start(
                            out[kt * PT:kt * PT + mo, js], osb[:mo, :])

                    for kt in (0, CT - 1) + tuple(range(1, CT - 1)):
                        mk = 33 if kt == CT - 1 else PT
                        ps_o = psO.tile([PT, JB], F32, tag="o",
                                        name=f"o{jb}_{kt}")
                        for t in range(IT):
                            nc.tensor.matmul(
                                ps_o[:mk, :],
                                qt[:, t, kt * PT:kt * PT + mk],
                                es[:, t, :],
                                start=(t == 0),
                                stop=(t == IT - 1),
                                skip_group_check=True,
                            )
                        if kt == CT - 1:
                            # fp16 1/den costs ~5e-4 relative error on the
                            # output, far under the 2e-2 gate; halves SBUF
                            recip = rp.tile([1, JB], F16, tag="recip")
                            with nc.allow_low_precision(
                                    reason="fp16 softmax denom recip"):
                                nc.vector.reciprocal(
                                    recip[:], ps_o[32:33, :])
                            bc = rp.tile([PT, JB], F16, tag="bc")
                            nc.gpsimd.partition_broadcast(bc[:], recip[:])
                        if bc is None:
                            held = (kt, ps_o)
                            continue
                        normalize(kt, ps_o)
                        if held is not None:
                            normalize(*held)
                            held = None

            xtp.release()
            xfp.release()
            xqp.release()
            qtp.release()

    nc.compile()
    return nc


_CACHE = {}


def _get_program(P=4096, n_cores=8):
    key = (P, n_cores)
    if key not in _CACHE:
        _CACHE[key] = build(P, n_cores)
    return _CACHE[key]


def _run(inputs, trace=False, **kw):
    nc = _get_program()
    Xq = np.asarray(inputs["Xq"], dtype=np.float32)
    Xk = np.asarray(inputs["Xk"], dtype=np.float32)
    Wk = np.ascontiguousarray(np.asarray(inputs["Wk"], dtype=np.float32))
    bkv = np.ascontiguousarray(np.asarray(inputs["bk"], dtype=np.float32))
    Wv = np.ascontiguousarray(np.asarray(inputs["Wv"], dtype=np.float32))
    bvv = np.ascontiguousarray(np.asarray(inputs["bv"], dtype=np.float32))
    B = Xq.shape[0]
    in_maps = [
        {
            "Xq": np.ascontiguousarray(Xq[b]),
            "Xk": np.ascontiguousarray(Xk[b]),
            "Wk": Wk,
            "bk": bkv,
            "Wv": Wv,
            "bv": bvv,
        }
        for b in range(B)
    ]
    res = run_bass_kernel_spmd(nc, in_maps, list(range(B)), trace=trace, **kw)
    outs = np.stack([res.results[b]["out"] for b in range(B)], axis=0)
    return outs.astype(np.float32), res


def kernel(**inputs):
    outs, _ = _run(inputs)
    return outs


# revision 35
# speedup vs baseline: 1.0138x; 1.0099x over previous
"""CrossTransFormer attention kernel for 8x Trainium2 NeuronCores (Bass/Tile).

Problem (per batch b, B=8, C=773, P=4096):
    K = Wk @ Xk + bk            [C, P]
    V = Wv @ Xq + bv            [C, P]
    S[i, j] = sum_c K[c, i] * V[c, j] / sqrt(C)       (i, j over P)
    H = softmax(S, axis=i)
    out[k, j] = sum_i Xk[k, i] * H[i, j]              [C, P]

Sharding: data-parallel over batch, one batch per NeuronCore, no collectives.

Algebraic restructure (saves one full projection + all weight transposes):
    S = Xk^T (Wk^T Wv) Xq + u 1^T + 1 w^T   with u = Xk^T (Wk^T bv);
    the j-indexed w term is constant along the softmax axis i and cancels
    exactly -> dropped.
  GT = Wv^T Wk is computed on the PE with both weights in NATURAL layout,
  w1 = Wk^T bv rides along; both fold into the A-projection
  A = G Xq + w1 1^T.  The A-proj lhsT tiles are zero-padded to c1=896 so
  every projection chain emits full-128-partition PSUM tiles: the staged A
  is then zero-filled in its ragged rows FOR FREE, letting every S matmul
  run K=128.

QT = Xk^T is produced by the DMA XBAR transpose engine (14 batched
dma_start_transpose calls), entirely off the PE.  xk16 c-tile 6 carries an
all-ones row at partition 32, so the transpose plants an all-ones column at
qt col 800 for free; the ragged out-chain then lands softmax sums on PSUM
partition 32 (legal compute-engine base).  No plain SBUF->SBUF HWDGE DMAs
are issued anywhere (XBAR-transpose || SBUF->SBUF DMA is a known HW
deadlock): the w1 row is computed directly into PSUM partition 5 by giving
the bias column M=6 (cols 0..4 zero), and the xq6 ones row comes from a
memset-1.0 + partial overwrite.

Fused phase D (per j-block of 512), everything SBUF-resident:
  A-proj: 7 chains of 7 MMs -> ast[128, 7, 512] fp16 (no DRAM staging).
  S-phase: 32 i-tiles, 7-MM chains into triple-buffered PSUM, ACT exp
  (scale=1/sqrt(C)) into es[128, 32, 512] fp16.
  out-phase: 7 k-tile chains of 32 accumulating MMs; the ragged chain
  (5 data rows + softmax-sum row from the qt ones-column) runs FIRST so
  the reciprocal + partition-broadcast overlap the remaining chains;
  each chain is normalized (DVE) and DMA'd out as it finishes.
"""

import sys

sys.path.insert(0, "/opt/trn_rl_repo")

import numpy as np

import concourse.bacc as bacc
import concourse.mybir as mybir
import concourse.tile as tile
from concourse.bass_utils import run_bass_kernel_spmd

F32 = mybir.dt.float32
F16 = mybir.dt.float16

C = 773
PT = 128
CT = 7  # ceil(773 / 128) chunks of the channel dim
LC = C - (CT - 1) * PT  # 5 rows in the last chunk
JB = 512  # j-block width (one PSUM bank of fp32)
CW = CT * PT  # c1 padded to 896 for the zero-padded A-proj lhsT
QW = 6 * PT + 48  # qt width: 6 full c-tiles + 48-col XBAR tail block


def build(P=4096, n_cores=8):
    NJ = P // JB
    IT = P // PT
    SCALE = float(1.0 / np.sqrt(C))

    nc = bacc.Bacc("TRN2", target_bir_lowering=False, debug=False,
                   num_devices=n_cores)
    Xq = nc.dram_tensor("Xq", [C, P], F32, kind="ExternalInput")
    Xk = nc.dram_tensor("Xk", [C, P], F32, kind="ExternalInput")
    Wk = nc.dram_tensor("Wk", [C, C], F32, kind="ExternalInput")
    bk = nc.dram_tensor("bk", [C], F32, kind="ExternalInput")
    Wv = nc.dram_tensor("Wv", [C, C], F32, kind="ExternalInput")
    bv = nc.dram_tensor("bv", [C], F32, kind="ExternalInput")
    out = nc.dram_tensor("out", [C, P], F32, kind="ExternalOutput")
    del bk  # only enters via a softmax-invariant per-j term

    with tile.TileContext(nc) as tc:
        with tc.tile_pool(name="persist", bufs=1) as persist:
            # Xk fp16 resident, natural [c, p] layout: lhsT tiles for S.
            # Tile 6: rows 0..4 = ragged data, row 32 = all-ones (becomes
            # the qt ones-column via the XBAR transpose; contributes 0 to S
            # because ast tile-6 rows 5..127 are zero), rest zeros.
            xk16 = persist.tile([PT, CT, P], F16)
            # exp(S) for one j-block, [i-in-tile, it, j]
            es = persist.tile([PT, IT, JB], F16)
            # GT = Wv^T Wk [c2-part, ct2, c1] fp16, c1 zero-padded to 896
            g16 = persist.tile([PT, CT, CW], F16)
            # packed ragged lhsT: rows 0..4 = GT c2-ragged rows, row 5 = w1
            g6 = persist.tile([8, CW], F16)

            # PE warmup: dummy matmuls so the HAM clock-gate opens
            # (4/8 -> 8/8) while the first DMAs are in flight, and the
            # exp activation table loads before the main loop.  warm is
            # memset on DVE (gpsimd takes ~8us to boot).
            wsb = tc.alloc_tile_pool(name="wsb", bufs=1)
            warm = wsb.tile([PT, JB], F16)
            nc.vector.memset(warm[:, :], 0.0)
            with tc.tile_pool(name="pswarm", bufs=4, space="PSUM") as pswarm:
                for i in range(52):
                    wps = pswarm.tile([PT, JB], F32, tag="wps",
                                      name=f"wps{i}")
                    nc.tensor.matmul(wps[:, :], warm[:, :PT], warm[:, :],
                                     start=True, stop=True,
                                     skip_group_check=True)
                wexp = wsb.tile([1, 16], F32)
                nc.scalar.activation(wexp[:], wps[:1, :16],
                                     mybir.ActivationFunctionType.Exp,
                                     scale=1.0)
            wsb.release()

            # zero-pad fills on gpsimd (consumers run ~15us+, gpsimd boot
            # overlaps).  The xk16 tile-6 fills are issued on DVE but only
            # AFTER the W casts below, so the G-phase critical path is not
            # delayed; their first consumer is the jc-0 tail cast (~20us).
            nc.gpsimd.memset(g16[:, :, :], 0.0)
            nc.gpsimd.memset(g6[:, :], 0.0)

            # ---- Phase G: GT = Wv^T Wk and w1 = Wk^T bv on the PE ----
            with (
                tc.tile_pool(name="wstg", bufs=6) as wstg,
                tc.tile_pool(name="wtlp", bufs=2) as wtlp,
                tc.tile_pool(name="wload", bufs=1) as wload,
                tc.tile_pool(name="psg", bufs=4, space="PSUM") as psg,
            ):
                wk16 = wload.tile([PT, CT, C], F16, tag="wk16")
                wv16 = wload.tile([PT, CT, C], F16, tag="wv16")
                # bias columns, M=6 per o-tile: cols 0..4 zero, col 5 = bv
                # chunk -> the w1 chain emits w1 directly on PSUM row 5.
                bvcol = wload.tile([PT, CT, 6], F16, tag="bvcol")
                # per-chunk W loads through a 6-deep ring: 12 concurrent
                # DMAs pull ~250 GB/s aggregate (one queue sustains only
                # ~20 GB/s), landing W in ~12us so phase G starts early.
                for Wsrc, dst in ((Wk, wk16), (Wv, wv16)):
                    for ct in range(CT - 1):
                        ws = wstg.tile([PT, C], F32, tag="wstage")
                        nc.sync.dma_start(
                            ws[:, :], Wsrc[ct * PT:(ct + 1) * PT, :])
                        nc.vector.tensor_copy(dst[:, ct, :], ws[:, :])
                    wt = wtlp.tile([8, C], F32, tag="wtail")
                    nc.sync.dma_start(wt[:LC, :], Wsrc[(CT - 1) * PT:C, :])
                    nc.vector.tensor_copy(dst[:LC, CT - 1, :], wt[:LC, :])
                # deferred DVE fills (after the W casts in DVE program
                # order).  xk16 tile 6: rows 0..4 = ragged data (cast in
                # phase B), row 32 = all-ones -> qt ones-column via XBAR.
                nc.vector.memset(xk16[:, CT - 1, :], 0.0)
                nc.vector.memset(xk16[32:33, CT - 1, :], 1.0)
                nc.vector.memset(bvcol[:, :, :], 0.0)
                # bv chunks into bvcol[:, ot, 5] on the gpsimd software
                # queue (DRAM->SBUF, cast f32->f16)
                for ot in range(CT - 1):
                    nc.gpsimd.dma_start(
                        bvcol[:, ot, 5:6], bv[ot * PT:(ot + 1) * PT, None])
                nc.gpsimd.dma_start(bvcol[:LC, CT - 1, 5:6],
                                    bv[(CT - 1) * PT:C, None])
                # GT tiles: [c2-tile, c1-chunk], contract over o (7 tiles)
                for ct2 in range(CT):
                    pc2 = PT if ct2 < CT - 1 else LC
                    for h, (j0, j1) in enumerate(((0, JB), (JB, C))):
                        ps = psg.tile([PT, JB], F32, tag="psg")
                        for ot in range(CT):
                            po = PT if ot < CT - 1 else LC
                            nc.tensor.matmul(
                                ps[:pc2, :j1 - j0],
                                wv16[:po, ot, ct2 * PT:ct2 * PT + pc2],
                                wk16[:po, ot, j0:j1],
                                start=(ot == 0),
                                stop=(ot == CT - 1),
                            )
                        # evacuate on the idle ACT engine: DVE is busy
                        # with W/Xk casts and would stall the G chains
                        nc.scalar.activation(
                            g16[:pc2, ct2, j0:j1], ps[:pc2, :j1 - j0],
                            mybir.ActivationFunctionType.Copy, scale=1.0)
                # w1 row: lhsT = bvcol (M=6, cols 0..4 zero) -> psum rows
                # 0..4 zero, row 5 = w1.  Copy rows 0..5 into g6 FIRST,
                # then overwrite rows 0..4 with the GT ragged rows (WAW
                # dep keeps the order).
                for h, (j0, j1) in enumerate(((0, JB), (JB, C))):
                    ps = psg.tile([8, JB], F32, tag="psw")
                    for ot in range(CT):
                        po = PT if ot < CT - 1 else LC
                        nc.tensor.matmul(
                            ps[:6, :j1 - j0],
                            bvcol[:po, ot, :],
                            wk16[:po, ot, j0:j1],
                            start=(ot == 0),
                            stop=(ot == CT - 1),
                        )
                    nc.scalar.activation(
                        g6[:6, j0:j1], ps[:6, :j1 - j0],
                        mybir.ActivationFunctionType.Copy, scale=1.0)
                nc.scalar.activation(
                    g6[:LC, :C], g16[:LC, CT - 1, :C],
                    mybir.ActivationFunctionType.Copy, scale=1.0)

            # QT pool reuses the space wload released.  qt[i, it, c]:
            # cols 0..767 from c-tiles 0..5, cols 768..815 from the 48-row
            # tail block (data rows 0..4 -> cols 768..772, ones row 32 ->
            # col 800, zeros elsewhere).
            qtp = tc.alloc_tile_pool(name="qtp", bufs=1)
            qt = qtp.tile([PT, IT, QW], F16)

            # pools that span phases B and D.  xfp is a deep per-chunk
            # staging ring: input DMA throughput scales with the number of
            # in-flight dma_starts (~20 GB/s per queue), so 7 concurrent
            # 256 KB chunk loads pull ~2x the aggregate bandwidth of the
            # 2-deep batched scheme.
            xqp = tc.alloc_tile_pool(name="xqp", bufs=2)
            xfp = tc.alloc_tile_pool(name="xfp", bufs=7)
            xtp = tc.alloc_tile_pool(name="xtp", bufs=2)

            def load_xq(jb):
                js = slice(jb * JB, (jb + 1) * JB)
                xq16 = xqp.tile([PT, CT, JB], F16, tag="xq16",
                                name=f"xq16_{jb}")
                for ct in range(CT - 1):
                    xf = xfp.tile([PT, JB], F32, tag="xstage",
                                  name=f"xqf{jb}_{ct}")
                    nc.sync.dma_start(
                        xf[:, :], Xq[ct * PT:(ct + 1) * PT, js])
                    nc.vector.tensor_copy(xq16[:, ct, :], xf[:, :])
                xt = xtp.tile([8, JB], F32, tag="xtail", name=f"xqt{jb}")
                nc.sync.dma_start(xt[:LC, :], Xq[(CT - 1) * PT:C, js])
                nc.vector.tensor_copy(xq16[:LC, CT - 1, :], xt[:LC, :])
                # packed ragged rhs: memset 1.0 (row 5 stays ones, rows
                # 6..7 hit zero g6 rows), rows 0..4 overwritten with the
                # Xq c2-ragged rows.
                xq6 = xqp.tile([8, JB], F16, tag="xq6", name=f"xq6_{jb}")
                nc.vector.memset(xq6[:, :], 1.0)
                nc.vector.tensor_copy(xq6[:LC, :], xq16[:LC, CT - 1, :])
                return xq16, xq6

            # ---- Phase B: stream Xk -> resident fp16 (DMA + DVE only);
            # xq block 0 is interleaved after jc 0 so the Xk stream (the
            # long pole for S0) starts first. ----
            xq_next = None
            for jc in range(NJ):
                js = slice(jc * JB, (jc + 1) * JB)
                for ct in range(CT - 1):
                    xf = xfp.tile([PT, JB], F32, tag="xstage",
                                  name=f"xkf{jc}_{ct}")
                    nc.sync.dma_start(
                        xf[:, :], Xk[ct * PT:(ct + 1) * PT, js])
                    nc.vector.tensor_copy(xk16[:, ct, js], xf[:, :])
                xt = xtp.tile([8, JB], F32, tag="xtail", name=f"xkt{jc}")
                nc.sync.dma_start(xt[:LC, :], Xk[(CT - 1) * PT:C, js])
                nc.vector.tensor_copy(xk16[:LC, CT - 1, js], xt[:LC, :])
                if jc == 1:
                    xq_next = load_xq(0)

            # prefetch xq block 1 BEFORE the XBAR gens occupy the sync
            # queue (they block on cast semaphores until ~2/3 through
            # phase B, which would delay block 1's A-projection)
            xq_pre = load_xq(1)

            # ---- QT: batched XBAR transposes (off the PE entirely) ----
            # Two halves per c-tile so the first half can fire as soon as
            # jc 0..3 have landed.  out[p, t, f] = in[f, 128*t + p].
            for half in (0, 1):
                hp = slice(half * (P // 2), (half + 1) * (P // 2))
                ht = slice(half * (IT // 2), (half + 1) * (IT // 2))
                for ct in range(CT):
                    pc = PT if ct < CT - 1 else 48
                    nc.sync.dma_start_transpose(
                        qt[:, ht, ct * PT:ct * PT + pc],
                        xk16[:pc, ct, hp],
                    )

            # ---- Phase D: fused A-projection + attention main loop ----
            with (
                tc.tile_pool(name="astp", bufs=2) as astp,
                tc.tile_pool(name="op", bufs=2) as op,
                tc.tile_pool(name="rp", bufs=1) as rp,
                tc.tile_pool(name="psA", bufs=2, space="PSUM") as psA,
                tc.tile_pool(name="psS", bufs=3, space="PSUM") as psS,
                tc.tile_pool(name="psO", bufs=3, space="PSUM") as psO,
            ):
                for jb in range(NJ):
                    js = slice(jb * JB, (jb + 1) * JB)
                    xq16, xq6 = xq_next
                    if jb == 0:
                        xq_next = xq_pre
                    elif jb < NJ - 1:
                        xq_next = load_xq(jb + 1)

                    # A-proj: A[:, jblock] = G @ Xq + w1 (ragged K=6 MM
                    # carries both the c2 tail and the bias row); the
                    # zero-padded lhsT makes all 128 psum rows valid
                    ast = astp.tile([PT, CT, JB], F16, tag="ast",
                                    name=f"ast{jb}")
                    for ot in range(CT):
                        ps = psA.tile([PT, JB], F32, tag="a",
                                      name=f"a{jb}_{ot}")
                        for ct2 in range(CT - 1):
                            nc.tensor.matmul(
                                ps[:, :],
                                g16[:, ct2, ot * PT:(ot + 1) * PT],
                                xq16[:, ct2, :],
                                start=(ct2 == 0),
                                stop=False,
                                skip_group_check=True,
                            )
                        nc.tensor.matmul(
                            ps[:, :],
                            g6[:LC + 1, ot * PT:(ot + 1) * PT],
                            xq6[:LC + 1, :],
                            start=False,
                            stop=True,
                            skip_group_check=True,
                        )
                        nc.any.tensor_copy(ast[:, ot, :], ps[:, :])

                    # S-phase: 32 chains of 7 K=128 MMs, exp into es
                    for t in range(IT):
                        ts = slice(t * PT, (t + 1) * PT)
                        ps_s = psS.tile([PT, JB], F32, tag="s",
                                        name=f"s{jb}_{t}")
                        for ct in range(CT):
                            nc.tensor.matmul(
                                ps_s[:, :],
                                xk16[:, ct, ts],
                                ast[:, ct, :],
                                start=(ct == 0),
                                stop=(ct == CT - 1),
                                skip_group_check=True,
                            )
                        nc.scalar.activation(
                            es[:, t, :], ps_s[:],
                            mybir.ActivationFunctionType.Exp, scale=SCALE,
                        )

                    # out-phase: chain kt=0 FIRST (its early MMs only need
                    # the early es tiles, hiding the exp tail), then the
                    # ragged chain (data rows 768..772 + sums row from the
                    # qt ones column) so the reciprocal + broadcast still
                    # overlap the remaining chains.  kt=0's normalization
                    # is deferred until bc exists (PSUM bank held).
                    bc = None
                    held = None

                    def normalize(kt, ps_o):
                        mo = LC if kt == CT - 1 else PT
                        osb = op.tile([PT, JB], F32, tag="osb",
                                      name=f"osb{jb}_{kt}")
                        nc.vector.tensor_mul(
                            out=osb[:mo, :], in0=ps_o[:mo, :],
                            in1=bc[:mo, :],
                        )
                        nc.sync.dma_start(
                            out[kt * PT:kt * PT + mo, js], osb[:mo, :])

                    for kt in (0, CT - 1) + tuple(range(1, CT - 1)):
                        mk = 33 if kt == CT - 1 else PT
                        ps_o = psO.tile([PT, JB], F32, tag="o",
                                        name=f"o{jb}_{kt}")
                        for t in range(IT):
                            nc.tensor.matmul(
                                ps_o[:mk, :],
                                qt[:, t, kt * PT:kt * PT + mk],
                                es[:, t, :],
                                start=(t == 0),
                                stop=(t == IT - 1),
                                skip_group_check=True,
                            )
                        if kt == CT - 1:
                            # fp16 1/den costs ~5e-4 relative error on the
                            # output, far under the 2e-2 gate; halves SBUF
                            recip = rp.tile([1, JB], F16, tag="recip")
                            with nc.allow_low_precision(
                                    reason="fp16 softmax denom recip"):
                                nc.vector.reciprocal(
                                    recip[:], ps_o[32:33, :])
                            bc = rp.tile([PT, JB], F16, tag="bc")
                            nc.gpsimd.partition_broadcast(bc[:], recip[:])
                        if bc is None:
                            held = (kt, ps_o)
                            continue
                        normalize(kt, ps_o)
                        if held is not None:
                            normalize(*held)
                            held = None

            xtp.release()
            xfp.release()
            xqp.release()
            qtp.release()

    nc.compile()
    return nc


_CACHE = {}


def _get_program(P=4096, n_cores=8):
    key = (P, n_cores)
    if key not in _CACHE:
        _CACHE[key] = build(P, n_cores)
    return _CACHE[key]


def _run(inputs, trace=False, **kw):
    nc = _get_program()
    Xq = np.asarray(inputs["Xq"], dtype=np.float32)
    Xk = np.asarray(inputs["Xk"], dtype=np.float32)
    Wk = np.ascontiguousarray(np.asarray(inputs["Wk"], dtype=np.float32))
    bkv = np.ascontiguousarray(np.asarray(inputs["bk"], dtype=np.float32))
    Wv = np.ascontiguousarray(np.asarray(inputs["Wv"], dtype=np.float32))
    bvv = np.ascontiguousarray(np.asarray(inputs["bv"], dtype=np.float32))
    B = Xq.shape[0]
    in_maps = [
        {
            "Xq": np.ascontiguousarray(Xq[b]),
            "Xk": np.ascontiguousarray(Xk[b]),
            "Wk": Wk,
            "bk": bkv,
            "Wv": Wv,
            "bv": bvv,
        }
        for b in range(B)
    ]
    res = run_bass_kernel_spmd(nc, in_maps, list(range(B)), trace=trace, **kw)
    outs = np.stack([res.results[b]["out"] for b in range(B)], axis=0)
    return outs.astype(np.float32), res


def kernel(**inputs):
    outs, _ = _run(inputs)
    return outs
